# revision 27
# baseline (speedup 1.0000x reference)
"""Causal self-attention (B=2, T=2048, C=1024, nh=16) on 8 TRN2 NeuronCores.

Sharding: core c = 4*b + g handles batch b (2048 tokens) and head-group g
(4 heads).  Megatron-style: QKV rows and proj columns sharded by head group;
the proj partial sums are reduced on the host (the "all-reduce").

All matmul operands are bf16 (fp32 PSUM accumulate); DMA payloads are bf16.
Per-core kernel, single interleaved instruction stream:
  1. QKV projection per 512-token chunk: kT and q as [f, t] m-groups with
     k-outputs split into zero-padded per-parity buffers (full-K QK matmuls);
     v computed directly in [t, d] layout (x t-tile stationary, W_v moving)
     and scattered next to shared all-ones columns so the PV matmul also
     emits softmax row-sums pre-broadcast on the complement 64 partitions.
  2. attention per chunk/head: S.T = kT.T@q per s-tile pair into 2-bank
     psum, exp on ACT, 0/1 causal mask post-exp on gpsimd (diag blocks),
     PV accumulation v_aug.T @ P.T -> yT + rowsum.
  3. normalization: fast-approx reciprocal of the rowsums + DMA partition
     swap + PSUM-direct multiplies into ysb.
  4. proj per chunk; v-bias folded into the proj bias on the host
     (y = P(v+bv)/rowsum => bp_eff = bp + W_proj @ bv); output partials
     stored bf16, summed on host in fp32.
  Next-chunk QKV units and previous-chunk proj units are injected between
  attention steps so exp (ACT) never stalls the tensor-engine FIFO.
"""

import os
import numpy as np

B, T, C, NH, HD = 2, 2048, 1024, 16, 64
HPC = 4  # heads per core
NCORES = 8
NCH = 4       # 512-wide t-chunks
CHW = 512

_cache = {}


def _build_nc():
    from contextlib import ExitStack

    import concourse.bass as bass
    import concourse.tile as tile
    from concourse import bacc, mybir

    f32 = mybir.dt.float32
    bf16 = mybir.dt.bfloat16
    AF = mybir.ActivationFunctionType
    OP = mybir.AluOpType

    nc = bacc.Bacc("TRN2", target_bir_lowering=False, debug=False,
                   num_devices=NCORES)

    xt = nc.dram_tensor("xt", [C, T], bf16, kind="ExternalInput").ap()
    wkqv = nc.dram_tensor("wkqv", [C, 3 * HPC * HD], bf16,
                          kind="ExternalInput").ap()
    bkq = nc.dram_tensor("bkq", [128, 4], f32, kind="ExternalInput").ap()
    wproj = nc.dram_tensor("wproj", [HPC * HD, C], bf16,
                           kind="ExternalInput").ap()
    bp = nc.dram_tensor("bp", [128, 8], f32, kind="ExternalInput").ap()
    amask_d = nc.dram_tensor("amask", [128, 128], bf16,
                             kind="ExternalInput").ap()
    zpad_d = nc.dram_tensor("zpad", [64, 2 * T], bf16,
                            kind="ExternalInput").ap()
    ones_d = nc.dram_tensor("ones", [128, 4096], bf16,
                            kind="ExternalInput").ap()
    outp = nc.dram_tensor("outp", [C, T], bf16, kind="ExternalOutput").ap()
    # second partial for the last chunk's head-pair-1 proj half (tail split);
    # host adds it onto outp's last 512 columns
    outp2 = nc.dram_tensor("outp2", [C, CHW], bf16,
                           kind="ExternalOutput").ap()

    with tile.TileContext(nc) as tc, ExitStack() as ctx:
        sing = ctx.enter_context(tc.tile_pool(name="sing", bufs=1))
        xpool = ctx.enter_context(tc.tile_pool(name="xpool", bufs=2))
        ptp = ctx.enter_context(tc.tile_pool(name="ptp", bufs=3))
        nrm = ctx.enter_context(tc.tile_pool(name="nrm", bufs=2))
        osp = ctx.enter_context(tc.tile_pool(name="osp", bufs=3))
        ps = ctx.enter_context(tc.tile_pool(name="ps", bufs=1, space="PSUM"))

        # ---- resident SBUF tensors ----
        wk = sing.tile([128, 8, 768], bf16, name="wk")
        qsb = sing.tile([128, 2, T], bf16, name="qsb")
        # zero-padded kT for full-K QK matmuls: even heads in rows 0:64 of
        # ktp_e (rows 64:128 zero), odd heads in rows 64:128 of ktp_o
        ktp_e = sing.tile([128, 2, T], bf16, name="ktp_e")
        ktp_o = sing.tile([128, 2, T], bf16, name="ktp_o")
        # vsb: 32 blocks of [v_A(64) | ones(128) | v_B(64)]
        vsb = sing.tile([128, 32 * 256], bf16, name="vsb")
        ysb = sing.tile([128, 2, T], bf16, name="ysb")
        wp = sing.tile([128, 2, C], bf16, name="wp")
        bkq_s = sing.tile([128, 4], f32, name="bkq_s")
        bp_s = sing.tile([128, 8], f32, name="bp_s")
        amask = sing.tile([128, 128], bf16, name="amask")

        onesanchor = vsb[:, 64:65]
        ones_view = bass.AP(tensor=onesanchor.tensor, offset=onesanchor.offset,
                            ap=[onesanchor.ap[0], [256, 32], [1, 128]])

        # constants (ktp zero-pads, all-ones columns) come in via DMA, not
        # memset: memsets at the head of the DVE FIFO delay the first QKV
        # evacuations by ~9us.  per-k interleaved x/wk loads so the k-major
        # first QKV group can start after the first k-tile pair lands.
        xt_r = xt.rearrange("(kk p) t -> p kk t", p=128)
        wkqv_r = wkqv.rearrange("(kk p) f -> p kk f", p=128)
        xts_tiles = {}
        xts0 = xpool.tile([128, 8, CHW], bf16, name="xts")
        nc.sync.dma_start(bkq_s, bkq)
        for k in range(8):
            nc.scalar.dma_start(xts0[:, k, :], xt_r[:, k, 0:CHW])
            nc.sync.dma_start(wk[:, k, :], wkqv_r[:, k, :])
        xts_tiles[0] = xts0
        nc.scalar.dma_start(ktp_e[64:128, :, :], zpad_d)
        nc.scalar.dma_start(ktp_o[0:64, :, :], zpad_d)
        nc.scalar.dma_start(ones_view,
                            ones_d.rearrange("p (a b) -> p a b", a=32))
        nc.sync.dma_start(amask, amask_d)
        nc.sync.dma_start(bp_s, bp)
        xts1 = xpool.tile([128, 8, CHW], bf16, name="xts")
        for k in range(8):
            nc.scalar.dma_start(xts1[:, k, :], xt_r[:, k, CHW:2 * CHW])
        xts_tiles[1] = xts1
        nc.sync.dma_start(wp, wproj.rearrange("(kk p) f -> p kk f", p=128))

        def v_stationary(j, h):
            """[128,128] AP: even slot -> [v_A|ones64], odd -> [ones64|v_B]."""
            hf, sl = h // 2, h % 2
            off = (j * 2 + hf) * 256 + 128 * sl
            return vsb[:, off:off + 128]

        def prefetch_x(n):
            def emit():
                xts = xpool.tile([128, 8, CHW], bf16, name="xts")
                for k in range(8):
                    nc.sync.dma_start(xts[:, k, :],
                                      xt_r[:, k, n * CHW:(n + 1) * CHW])
                xts_tiles[n] = xts
            return emit

        def qkv_m_unit(n, m):
            """kT (m=0,1) / q (m=2,3) f-tile of chunk n: 8 matmuls + evac."""
            def emit():
                with nc.named_scope(f"qkv{n}"):
                    xts = xts_tiles[n]
                    cols = slice(n * CHW, (n + 1) * CHW)
                    acc = ps.tile([128, CHW], f32, name="acc", tag="acc",
                                  bufs=2)
                    for k in range(8):
                        nc.tensor.matmul(
                            acc, wk[:, k, m * 128:(m + 1) * 128], xts[:, k, :],
                            start=(k == 0), stop=(k == 7))
                    if m < 2:
                        nc.vector.tensor_scalar_add(
                            out=ktp_e[0:64, m, cols], in0=acc[0:64, :],
                            scalar1=bkq_s[0:64, m:m + 1])
                        nc.vector.tensor_scalar_add(
                            out=ktp_o[64:128, m, cols], in0=acc[64:128, :],
                            scalar1=bkq_s[64:128, m:m + 1])
                    else:
                        nc.vector.tensor_scalar_add(
                            out=qsb[:, m - 2, cols], in0=acc,
                            scalar1=bkq_s[:, m:m + 1])
            return emit

        def qkv_v_unit(n, p):
            """vT for t-tiles 2p,2p+1 of chunk n, direct [t, d] layout."""
            def emit():
                with nc.named_scope(f"qkv{n}"):
                    xts = xts_tiles[n]
                    acc = ps.tile([128, CHW], f32, name="acc", tag="acc",
                                  bufs=2)
                    for t2 in range(2):
                        tt = 2 * p + t2
                        for k in range(8):
                            nc.tensor.matmul(
                                acc[:, t2 * 256:(t2 + 1) * 256],
                                xts[:, k, tt * 128:(tt + 1) * 128],
                                wk[:, k, 512:768],
                                start=(k == 0), stop=(k == 7))
                    for t2 in range(2):
                        j = 4 * n + 2 * p + t2
                        anch = vsb[:, (2 * j) * 256:(2 * j) * 256 + 1]
                        dst = bass.AP(tensor=anch.tensor, offset=anch.offset,
                                      ap=[anch.ap[0], [256, 2], [192, 2],
                                          [1, 64]])
                        nc.vector.tensor_copy(
                            dst,
                            acc[:, t2 * 256:(t2 + 1) * 256].rearrange(
                                "p (hf sl d) -> p hf sl d", hf=2, sl=2))
            return emit

        def proj_unit(n, os_, act_bias=False):
            """proj o-tiles os_ over chunk n's 512 t-columns + bias + store."""
            def emit():
                with nc.named_scope(f"proj{n}"):
                    for o in os_:
                        acc = ps.tile([128, CHW], f32, name="acc", tag="acc",
                                      bufs=2)
                        for kk in range(2):
                            nc.tensor.matmul(
                                acc, wp[:, kk, o * 128:(o + 1) * 128],
                                ysb[:, kk, n * CHW:(n + 1) * CHW],
                                start=(kk == 0), stop=(kk == 1))
                        ot = osp.tile([128, CHW], bf16, name="ot")
                        if act_bias and o % 2:
                            nc.scalar.activation(out=ot, in_=acc,
                                                 func=AF.Identity,
                                                 bias=bp_s[:, o:o + 1])
                        else:
                            nc.vector.tensor_scalar_add(
                                out=ot, in0=acc, scalar1=bp_s[:, o:o + 1])
                        nc.sync.dma_start(
                            outp[o * 128:(o + 1) * 128,
                                 n * CHW:(n + 1) * CHW], ot)
            return emit

        def normalize(n, hf, pys):
            """yT = pvT * (1/rowsum), rowsums partition-swapped via DMA.

            The two full-width copies release the py PSUM banks as fast as
            possible (the next head-pair's first PV has a WAR on them);
            recip/swap/mult then run from SBUF off that critical path."""
            u0 = nrm.tile([128, CHW], f32, name="u0")
            u1 = nrm.tile([128, CHW], f32, name="u1")
            rb = nrm.tile([128, CHW], f32, name="rb")
            nc.vector.tensor_copy(u0, pys[0])
            nc.vector.tensor_copy(u1, pys[1])
            # swap the RAW rowsums first, then one full-128-partition
            # reciprocal (the custom DVE op misbehaves on partition-offset
            # slices)
            nc.sync.dma_start(rb[0:64, :], u0[64:128, :])
            nc.sync.dma_start(rb[64:128, :], u1[0:64, :])
            nc.vector.reciprocal_approx_fast(out=rb, in_=rb)
            cols = slice(n * CHW, (n + 1) * CHW)
            nc.vector.tensor_tensor(out=ysb[0:64, hf, cols],
                                    in0=u0[0:64, :], in1=rb[0:64, :],
                                    op=OP.mult)
            nc.vector.tensor_tensor(out=ysb[64:128, hf, cols],
                                    in0=u1[64:128, :], in1=rb[64:128, :],
                                    op=OP.mult)

        def attn_chunk(n, inject):
            """Attention for chunk n; inject (phase, fn) units between
            (a, sl) steps.  phase 1 units wait for hf=0's normalize."""
            steps = 4 * (2 * n + 2) + 2  # + one pump slot per normalize
            step = 0
            done = 0
            phase = 0
            jmax = 4 * n + 3

            def pump():
                nonlocal done
                quota = min(len(inject),
                            (len(inject) * step + steps - 1) // steps)
                while (done < quota and inject[done][0] <= phase):
                    inject[done][1]()
                    done += 1

            with nc.named_scope(f"attn{n}"):
                for hf in range(2):
                    pys = [
                        ps.tile([128, CHW], f32, name=f"py{sl}", tag="py",
                                bufs=2)
                        for sl in range(2)]
                    pend = []
                    for a in range(2 * n + 2):
                        for sl in range(2):
                            h = 2 * hf + sl
                            ktp = ktp_e if sl == 0 else ktp_o
                            ss = ps.tile([128, 2, CHW], f32, name="ss",
                                         tag="ss", bufs=2)
                            c0s = []
                            for idx in range(2):
                                j = 2 * a + idx
                                c0 = max(0, 128 * j - CHW * n)
                                c0s.append(c0)
                                nc.tensor.matmul(
                                    ss[:, idx, c0:],
                                    ktp[:, hf, j * 128:(j + 1) * 128],
                                    qsb[:, hf, n * CHW + c0:(n + 1) * CHW],
                                    start=True, stop=True)
                            for fn in pend:
                                fn()
                            pend = []
                            pt = ptp.tile([128, 2, CHW], bf16, name="pt")
                            # cols < c0 of diagonal tiles hold stale PSUM
                            # bits; exp of those lands in pt cols the PV
                            # matmul never reads, so a full-tile exp is safe.
                            # Merge when the skipped cols cost less than the
                            # ~352-cycle per-call ACT overhead, else split.
                            if c0s[0] + c0s[1] <= 352:
                                nc.scalar.activation(out=pt, in_=ss,
                                                     func=AF.Exp)
                            else:
                                for idx in range(2):
                                    c0 = c0s[idx]
                                    nc.scalar.activation(
                                        out=pt[:, idx, c0:],
                                        in_=ss[:, idx, c0:], func=AF.Exp)
                            for idx in range(2):
                                j = 2 * a + idx
                                c0 = c0s[idx]
                                if j >= 4 * n:  # diagonal: 0/1 mask post-exp
                                    nc.gpsimd.tensor_tensor(
                                        out=pt[:, idx, c0:c0 + 128],
                                        in0=pt[:, idx, c0:c0 + 128],
                                        in1=amask, op=OP.mult)

                            def mk_pv(a=a, c0s=c0s, pt=pt, h=h, py=pys[sl]):
                                for idx in range(2):
                                    j = 2 * a + idx
                                    c0 = c0s[idx]
                                    nc.tensor.matmul(
                                        py[:, c0:], v_stationary(j, h),
                                        pt[:, idx, c0:],
                                        start=(j == 0), stop=(j == jmax),
                                        skip_group_check=True)
                            pend.append(mk_pv)
                            step += 1
                            pump()
                    for fn in pend:
                        fn()
                    normalize(n, hf, pys)
                    phase = hf + 1
                    step += 1
                    pump()
            while done < len(inject):
                inject[done][1]()
                done += 1

        # ---- emission: k-major QKV(0) upfront (consumes per-k DMA pairs as
        # they land), then attn(n) with injected QKV(n+1) + proj(n-1) ----
        with nc.named_scope("qkv0"):
            acc_m = [ps.tile([128, CHW], f32, name=f"am{m}", tag=tg, bufs=2)
                     for m, tg in enumerate(("acc", "acc", "py", "py"))]
            accv = ps.tile([128, 2, CHW], f32, name="accv", tag="ss", bufs=2)
            for k in range(8):
                for m in range(4):
                    nc.tensor.matmul(
                        acc_m[m], wk[:, k, m * 128:(m + 1) * 128],
                        xts0[:, k, :], start=(k == 0), stop=(k == 7))
                for tt in range(4):
                    c0 = (tt % 2) * 256
                    # start=True clears has_written for the WHOLE bank, so
                    # only the first group per bank may start; the second
                    # group's k==0 write relies on that bank-wide clear
                    # (bit unset -> overwrite) and must never re-start.
                    nc.tensor.matmul(
                        accv[:, tt // 2, c0:c0 + 256],
                        xts0[:, k, tt * 128:(tt + 1) * 128],
                        wk[:, k, 512:768],
                        start=(k == 0 and tt % 2 == 0), stop=(k == 7),
                        skip_group_check=True)
            cols = slice(0, CHW)
            for m in (0, 2, 1, 3):  # attn0's first QK needs m0 (kT) + m2 (q)
                if m < 2:
                    nc.vector.tensor_scalar_add(
                        out=ktp_e[0:64, m, cols], in0=acc_m[m][0:64, :],
                        scalar1=bkq_s[0:64, m:m + 1])
                    nc.vector.tensor_scalar_add(
                        out=ktp_o[64:128, m, cols], in0=acc_m[m][64:128, :],
                        scalar1=bkq_s[64:128, m:m + 1])
                else:
                    nc.vector.tensor_scalar_add(
                        out=qsb[:, m - 2, cols], in0=acc_m[m],
                        scalar1=bkq_s[:, m:m + 1])
            for tt in range(4):
                anch = vsb[:, (2 * tt) * 256:(2 * tt) * 256 + 1]
                dst = bass.AP(tensor=anch.tensor, offset=anch.offset,
                              ap=[anch.ap[0], [256, 2], [192, 2], [1, 64]])
                c0 = (tt % 2) * 256
                nc.vector.tensor_copy(
                    dst, accv[:, tt // 2, c0:c0 + 256].rearrange(
                        "p (hf sl d) -> p hf sl d", hf=2, sl=2))

        def proj3_half(kk, os_):
            """Last chunk's proj split by head-pair: kk=0 (with bias) into
            outp as soon as hf=0 is normalized, kk=1 partial into outp2."""
            def emit():
                n = NCH - 1
                with nc.named_scope(f"proj{n}"):
                    for i, o in enumerate(os_):
                        tg = ("acc", "py")[i % 2] if kk == 1 else "acc"
                        acc = ps.tile([128, CHW], f32, name="acc", tag=tg,
                                      bufs=2)
                        nc.tensor.matmul(
                            acc, wp[:, kk, o * 128:(o + 1) * 128],
                            ysb[:, kk, n * CHW:(n + 1) * CHW],
                            start=True, stop=True)
                        ot = osp.tile([128, CHW], bf16, name="ot")
                        if kk == 0:
                            nc.vector.tensor_scalar_add(
                                out=ot, in0=acc, scalar1=bp_s[:, o:o + 1])
                            nc.sync.dma_start(
                                outp[o * 128:(o + 1) * 128,
                                     n * CHW:(n + 1) * CHW], ot)
                        else:
                            # alternate evac engines so the tail drains in
                            # parallel instead of serializing on ACT
                            if o % 2:
                                nc.scalar.copy(out=ot, in_=acc)
                            else:
                                nc.vector.tensor_copy(ot, acc)
                            nc.sync.dma_start(
                                outp2[o * 128:(o + 1) * 128, :], ot)
            return emit

        for n in range(NCH):
            inject = []
            if n >= 1:
                inject.append((0, proj_unit(n - 1, range(0, 4))))
                inject.append((0, proj_unit(n - 1, range(4, 8))))
            if n + 1 < NCH:
                inject.extend((0, qkv_m_unit(n + 1, m)) for m in range(2))
                if n + 2 < NCH:
                    inject.append((0, prefetch_x(n + 2)))
                inject.extend((0, qkv_m_unit(n + 1, m)) for m in range(2, 4))
                inject.extend((0, qkv_v_unit(n + 1, p)) for p in range(2))
            if n == NCH - 1:
                # last proj(2) unit reserved for the post-normalize(3,hf1)
                # window: it fills the PE gap while proj3b waits on ysb and
                # keeps the HAM clock from throttling through the tail
                inject = [inject[0], (1, proj3_half(0, range(0, 4))),
                          (1, proj3_half(0, range(4, 8))),
                          (2, inject[1][1])]
            attn_chunk(n, inject)

        proj3_half(1, range(8))()

    nc.compile()
    return nc


def _host_inputs(x, W_kqv, b_kqv, W_proj, b_proj):
    import ml_dtypes

    bf16 = ml_dtypes.bfloat16
    x = np.asarray(x, dtype=np.float32)
    W_kqv = np.asarray(W_kqv, dtype=np.float32)
    b_kqv = np.asarray(b_kqv, dtype=np.float32)
    W_proj = np.asarray(W_proj, dtype=np.float32)
    b_proj = np.asarray(b_proj, dtype=np.float32)

    ss, tt = np.meshgrid(np.arange(128), np.arange(128), indexing="ij")
    amask = (ss <= tt).astype(bf16)  # 0/1 multiplicative mask
    zpad = np.zeros((64, 2 * T), dtype=bf16)
    ones = np.ones((128, 4096), dtype=bf16)

    xts = [np.ascontiguousarray(x[b].T).astype(bf16) for b in range(B)]

    in_maps = []
    for c in range(NCORES):
        b, g = c // 4, c % 4
        heads = [4 * g + i for i in range(HPC)]
        wl = np.concatenate(
            [W_kqv[h * 192:h * 192 + 64] for h in heads]
            + [W_kqv[h * 192 + 64:h * 192 + 128] * 0.125 for h in heads]
            + [W_kqv[h * 192 + 128:h * 192 + 192] for h in heads], axis=0)
        bkql = np.concatenate(
            [b_kqv[h * 192:h * 192 + 64] for h in heads]
            + [b_kqv[h * 192 + 64:h * 192 + 128] * 0.125 for h in heads])
        wp_g = W_proj[:, 256 * g:256 * (g + 1)]
        bv_g = np.concatenate(
            [b_kqv[h * 192 + 128:h * 192 + 192] for h in heads])
        bp_eff = (b_proj if g == 0 else np.zeros_like(b_proj)) + wp_g @ bv_g
        in_maps.append({
            "xt": xts[b],
            "wkqv": np.ascontiguousarray(wl.T).astype(bf16),
            "bkq": np.ascontiguousarray(bkql.reshape(4, 128).T,
                                        dtype=np.float32),
            "wproj": np.ascontiguousarray(wp_g.T).astype(bf16),
            "bp": np.ascontiguousarray(bp_eff.reshape(8, 128).T,
                                       dtype=np.float32),
            "amask": amask,
            "zpad": zpad,
            "ones": ones,
        })
    return in_maps


def kernel(x, W_kqv, b_kqv, W_proj, b_proj):
    from concourse.bass_utils import run_bass_kernel_spmd

    if "nc" not in _cache:
        _cache["nc"] = _build_nc()
    nc = _cache["nc"]

    in_maps = _host_inputs(x, W_kqv, b_kqv, W_proj, b_proj)
    trace = bool(int(os.environ.get("KERNEL_TRACE", "0")))
    r = run_bass_kernel_spmd(nc, in_maps, core_ids=list(range(NCORES)),
                             trace=trace)
    if trace:
        _cache["last_results"] = r
        print(f"HW exec time: {r.exec_time_ns} ns")

    out = np.empty((B, T, C), dtype=np.float32)
    for b in range(B):
        acc = np.zeros((C, T), dtype=np.float32)
        for g in range(4):
            acc += r.results[4 * b + g]["outp"].astype(np.float32)
            acc[:, T - CHW:] += r.results[4 * b + g]["outp2"].astype(
                np.float32)
        out[b] = acc.T
    return out


# revision 31
# speedup vs baseline: 1.0432x; 1.0432x over previous
"""Causal self-attention (B=2, T=2048, C=1024, nh=16) on 8 TRN2 NeuronCores.

Sharding: core c = 4*b + g handles batch b (2048 tokens) and head-group g
(4 heads).  Megatron-style: QKV rows and proj columns sharded by head group;
the proj partial sums are reduced on the host (the "all-reduce").

All matmul operands are bf16 (fp32 PSUM accumulate); DMA payloads are bf16.
Per-core kernel, single interleaved instruction stream:
  1. QKV projection per 512-token chunk: kT and q as [f, t] m-groups with
     k-outputs split into zero-padded per-parity buffers (full-K QK matmuls);
     v computed directly in [t, d] layout (x t-tile stationary, W_v moving)
     and scattered next to shared all-ones columns so the PV matmul also
     emits softmax row-sums pre-broadcast on the complement 64 partitions.
  2. attention per chunk/head: S.T = kT.T@q per s-tile pair into 2-bank
     psum, exp on ACT, 0/1 causal mask post-exp on gpsimd (diag blocks),
     PV accumulation v_aug.T @ P.T -> yT + rowsum.
  3. normalization: fast-approx reciprocal of the rowsums + DMA partition
     swap + PSUM-direct multiplies into ysb.
  4. proj per chunk; v-bias folded into the proj bias on the host
     (y = P(v+bv)/rowsum => bp_eff = bp + W_proj @ bv); output partials
     stored bf16, summed on host in fp32.
  Next-chunk QKV units and previous-chunk proj units are injected between
  attention steps so exp (ACT) never stalls the tensor-engine FIFO.
"""

import os
import numpy as np

B, T, C, NH, HD = 2, 2048, 1024, 16, 64
HPC = 4  # heads per core
NCORES = 8
NCH = 4       # 512-wide t-chunks
CHW = 512

_cache = {}


def _build_nc():
    from contextlib import ExitStack

    import concourse.bass as bass
    import concourse.tile as tile
    from concourse import bacc, mybir

    f32 = mybir.dt.float32
    bf16 = mybir.dt.bfloat16
    AF = mybir.ActivationFunctionType
    OP = mybir.AluOpType

    nc = bacc.Bacc("TRN2", target_bir_lowering=False, debug=False,
                   num_devices=NCORES)

    xt = nc.dram_tensor("xt", [C, T], bf16, kind="ExternalInput").ap()
    wkqv = nc.dram_tensor("wkqv", [C, 3 * HPC * HD], bf16,
                          kind="ExternalInput").ap()
    bkq = nc.dram_tensor("bkq", [128, 4], f32, kind="ExternalInput").ap()
    wproj = nc.dram_tensor("wproj", [HPC * HD, C], bf16,
                           kind="ExternalInput").ap()
    bp = nc.dram_tensor("bp", [128, 8], f32, kind="ExternalInput").ap()
    amask_d = nc.dram_tensor("amask", [128, 128], bf16,
                             kind="ExternalInput").ap()
    zpad_d = nc.dram_tensor("zpad", [64, 2 * T], bf16,
                            kind="ExternalInput").ap()
    ones_d = nc.dram_tensor("ones", [128, 4096], bf16,
                            kind="ExternalInput").ap()
    outp = nc.dram_tensor("outp", [C, T], bf16, kind="ExternalOutput").ap()
    # second partial for the last chunk's head-pair-1 proj half (tail split);
    # host adds it onto outp's last 512 columns
    outp2 = nc.dram_tensor("outp2", [C, CHW], bf16,
                           kind="ExternalOutput").ap()

    with tile.TileContext(nc) as tc, ExitStack() as ctx:
        sing = ctx.enter_context(tc.tile_pool(name="sing", bufs=1))
        xpool = ctx.enter_context(tc.tile_pool(name="xpool", bufs=2))
        ptp = ctx.enter_context(tc.tile_pool(name="ptp", bufs=3))
        nrm = ctx.enter_context(tc.tile_pool(name="nrm", bufs=2))
        osp = ctx.enter_context(tc.tile_pool(name="osp", bufs=6))
        ps = ctx.enter_context(tc.tile_pool(name="ps", bufs=1, space="PSUM"))

        # ---- resident SBUF tensors ----
        wk = sing.tile([128, 8, 768], bf16, name="wk")
        qsb = sing.tile([128, 2, T], bf16, name="qsb")
        # zero-padded kT for full-K QK matmuls: even heads in rows 0:64 of
        # ktp_e (rows 64:128 zero), odd heads in rows 64:128 of ktp_o
        ktp_e = sing.tile([128, 2, T], bf16, name="ktp_e")
        ktp_o = sing.tile([128, 2, T], bf16, name="ktp_o")
        # vsb: 32 blocks of [v_A(64) | ones(128) | v_B(64)]
        vsb = sing.tile([128, 32 * 256], bf16, name="vsb")
        ysb = sing.tile([128, 2, T], bf16, name="ysb")
        wp = sing.tile([128, 2, C], bf16, name="wp")
        bkq_s = sing.tile([128, 4], f32, name="bkq_s")
        bp_s = sing.tile([128, 8], f32, name="bp_s")
        amask = sing.tile([128, 128], bf16, name="amask")

        onesanchor = vsb[:, 64:65]
        ones_view = bass.AP(tensor=onesanchor.tensor, offset=onesanchor.offset,
                            ap=[onesanchor.ap[0], [256, 32], [1, 128]])

        # constants (ktp zero-pads, all-ones columns) come in via DMA, not
        # memset: memsets at the head of the DVE FIFO delay the first QKV
        # evacuations by ~9us.  per-k interleaved x/wk loads so the k-major
        # first QKV group can start after the first k-tile pair lands.
        xt_r = xt.rearrange("(kk p) t -> p kk t", p=128)
        wkqv_r = wkqv.rearrange("(kk p) f -> p kk f", p=128)
        xts_tiles = {}
        xts0 = xpool.tile([128, 8, CHW], bf16, name="xts")
        nc.sync.dma_start(bkq_s, bkq)
        for k in range(8):
            nc.scalar.dma_start(xts0[:, k, :], xt_r[:, k, 0:CHW])
            nc.sync.dma_start(wk[:, k, :], wkqv_r[:, k, :])
        xts_tiles[0] = xts0
        nc.scalar.dma_start(ktp_e[64:128, :, :], zpad_d)
        nc.scalar.dma_start(ktp_o[0:64, :, :], zpad_d)
        nc.scalar.dma_start(ones_view,
                            ones_d.rearrange("p (a b) -> p a b", a=32))
        nc.sync.dma_start(amask, amask_d)
        nc.sync.dma_start(bp_s, bp)
        xts1 = xpool.tile([128, 8, CHW], bf16, name="xts")
        for k in range(8):
            nc.scalar.dma_start(xts1[:, k, :], xt_r[:, k, CHW:2 * CHW])
        xts_tiles[1] = xts1
        nc.sync.dma_start(wp, wproj.rearrange("(kk p) f -> p kk f", p=128))

        def v_stationary(j, h):
            """[128,128] AP: even slot -> [v_A|ones64], odd -> [ones64|v_B]."""
            hf, sl = h // 2, h % 2
            off = (j * 2 + hf) * 256 + 128 * sl
            return vsb[:, off:off + 128]

        def prefetch_x(n):
            def emit():
                xts = xpool.tile([128, 8, CHW], bf16, name="xts")
                for k in range(8):
                    nc.sync.dma_start(xts[:, k, :],
                                      xt_r[:, k, n * CHW:(n + 1) * CHW])
                xts_tiles[n] = xts
            return emit

        def qkv_m_unit(n, m):
            """kT (m=0,1) / q (m=2,3) f-tile of chunk n: 8 matmuls + evac."""
            def emit():
                with nc.named_scope(f"qkv{n}"):
                    xts = xts_tiles[n]
                    cols = slice(n * CHW, (n + 1) * CHW)
                    acc = ps.tile([128, CHW], f32, name="acc", tag="acc",
                                  bufs=2)
                    for k in range(8):
                        nc.tensor.matmul(
                            acc, wk[:, k, m * 128:(m + 1) * 128], xts[:, k, :],
                            start=(k == 0), stop=(k == 7))
                    if m < 2:
                        nc.vector.tensor_scalar_add(
                            out=ktp_e[0:64, m, cols], in0=acc[0:64, :],
                            scalar1=bkq_s[0:64, m:m + 1])
                        nc.vector.tensor_scalar_add(
                            out=ktp_o[64:128, m, cols], in0=acc[64:128, :],
                            scalar1=bkq_s[64:128, m:m + 1])
                    else:
                        nc.vector.tensor_scalar_add(
                            out=qsb[:, m - 2, cols], in0=acc,
                            scalar1=bkq_s[:, m:m + 1])
            return emit

        def qkv_v_unit(n, p):
            """vT for t-tiles 2p,2p+1 of chunk n, direct [t, d] layout."""
            def emit():
                with nc.named_scope(f"qkv{n}"):
                    xts = xts_tiles[n]
                    acc = ps.tile([128, CHW], f32, name="acc", tag="acc",
                                  bufs=2)
                    for t2 in range(2):
                        tt = 2 * p + t2
                        for k in range(8):
                            nc.tensor.matmul(
                                acc[:, t2 * 256:(t2 + 1) * 256],
                                xts[:, k, tt * 128:(tt + 1) * 128],
                                wk[:, k, 512:768],
                                start=(k == 0), stop=(k == 7))
                    for t2 in range(2):
                        j = 4 * n + 2 * p + t2
                        anch = vsb[:, (2 * j) * 256:(2 * j) * 256 + 1]
                        dst = bass.AP(tensor=anch.tensor, offset=anch.offset,
                                      ap=[anch.ap[0], [256, 2], [192, 2],
                                          [1, 64]])
                        nc.vector.tensor_copy(
                            dst,
                            acc[:, t2 * 256:(t2 + 1) * 256].rearrange(
                                "p (hf sl d) -> p hf sl d", hf=2, sl=2))
            return emit

        def proj_unit(n, os_, act_bias=False):
            """proj o-tiles os_ over chunk n's 512 t-columns + bias + store."""
            def emit():
                with nc.named_scope(f"proj{n}"):
                    for o in os_:
                        acc = ps.tile([128, CHW], f32, name="acc", tag="acc",
                                      bufs=2)
                        for kk in range(2):
                            nc.tensor.matmul(
                                acc, wp[:, kk, o * 128:(o + 1) * 128],
                                ysb[:, kk, n * CHW:(n + 1) * CHW],
                                start=(kk == 0), stop=(kk == 1))
                        ot = osp.tile([128, CHW], bf16, name="ot")
                        if act_bias and o % 2:
                            nc.scalar.activation(out=ot, in_=acc,
                                                 func=AF.Identity,
                                                 bias=bp_s[:, o:o + 1])
                        else:
                            nc.vector.tensor_scalar_add(
                                out=ot, in0=acc, scalar1=bp_s[:, o:o + 1])
                        nc.sync.dma_start(
                            outp[o * 128:(o + 1) * 128,
                                 n * CHW:(n + 1) * CHW], ot)
            return emit

        def normalize(n, hf, pys):
            """yT = pvT * (1/rowsum), rowsums partition-swapped via DMA.

            The two full-width copies release the py PSUM banks as fast as
            possible (the next head-pair's first PV has a WAR on them);
            recip/swap/mult then run from SBUF off that critical path."""
            u0 = nrm.tile([128, CHW], f32, name="u0")
            u1 = nrm.tile([128, CHW], f32, name="u1")
            rb = nrm.tile([128, CHW], f32, name="rb")
            nc.vector.tensor_copy(u0, pys[0])
            nc.vector.tensor_copy(u1, pys[1])
            # swap the RAW rowsums first, then one full-128-partition
            # reciprocal (the custom DVE op misbehaves on partition-offset
            # slices)
            nc.sync.dma_start(rb[0:64, :], u0[64:128, :])
            nc.scalar.dma_start(rb[64:128, :], u1[0:64, :])
            nc.vector.reciprocal_approx_fast(out=rb, in_=rb)
            cols = slice(n * CHW, (n + 1) * CHW)
            nc.vector.tensor_tensor(out=ysb[0:64, hf, cols],
                                    in0=u0[0:64, :], in1=rb[0:64, :],
                                    op=OP.mult)
            nc.vector.tensor_tensor(out=ysb[64:128, hf, cols],
                                    in0=u1[64:128, :], in1=rb[64:128, :],
                                    op=OP.mult)

        def attn_chunk(n, inject):
            """Attention for chunk n; inject (phase, fn) units between
            (a, sl) steps.  phase 1 units wait for hf=0's normalize."""
            steps = 4 * (2 * n + 2) + 2  # + one pump slot per normalize
            step = 0
            done = 0
            phase = 0
            jmax = 4 * n + 3

            def pump():
                nonlocal done
                quota = min(len(inject),
                            (len(inject) * step + steps - 1) // steps)
                while (done < quota and inject[done][0] <= phase):
                    inject[done][1]()
                    done += 1

            with nc.named_scope(f"attn{n}"):
                for hf in range(2):
                    pys = [
                        ps.tile([128, CHW], f32, name=f"py{sl}", tag="py",
                                bufs=2)
                        for sl in range(2)]
                    pend = []
                    for a in range(2 * n + 2):
                        for sl in range(2):
                            h = 2 * hf + sl
                            ktp = ktp_e if sl == 0 else ktp_o
                            ss = ps.tile([128, 2, CHW], f32, name="ss",
                                         tag="ss", bufs=2)
                            c0s = []
                            for idx in range(2):
                                j = 2 * a + idx
                                c0 = max(0, 128 * j - CHW * n)
                                c0s.append(c0)
                                nc.tensor.matmul(
                                    ss[:, idx, c0:],
                                    ktp[:, hf, j * 128:(j + 1) * 128],
                                    qsb[:, hf, n * CHW + c0:(n + 1) * CHW],
                                    start=True, stop=True)
                            for fn in pend:
                                fn()
                            pend = []
                            pt = ptp.tile([128, 2, CHW], bf16, name="pt")
                            # cols < c0 of diagonal tiles hold stale PSUM
                            # bits; exp of those lands in pt cols the PV
                            # matmul never reads, so a full-tile exp is safe.
                            # Merge when the skipped cols cost less than the
                            # ~352-cycle per-call ACT overhead, else split.
                            if c0s[0] + c0s[1] <= 352:
                                nc.scalar.activation(out=pt, in_=ss,
                                                     func=AF.Exp)
                            else:
                                for idx in range(2):
                                    c0 = c0s[idx]
                                    nc.scalar.activation(
                                        out=pt[:, idx, c0:],
                                        in_=ss[:, idx, c0:], func=AF.Exp)
                            for idx in range(2):
                                j = 2 * a + idx
                                c0 = c0s[idx]
                                if j >= 4 * n:  # diagonal: 0/1 mask post-exp
                                    nc.gpsimd.tensor_tensor(
                                        out=pt[:, idx, c0:c0 + 128],
                                        in0=pt[:, idx, c0:c0 + 128],
                                        in1=amask, op=OP.mult)

                            def mk_pv(a=a, c0s=c0s, pt=pt, h=h, py=pys[sl]):
                                for idx in range(2):
                                    j = 2 * a + idx
                                    c0 = c0s[idx]
                                    nc.tensor.matmul(
                                        py[:, c0:], v_stationary(j, h),
                                        pt[:, idx, c0:],
                                        start=(j == 0), stop=(j == jmax),
                                        skip_group_check=True)
                            pend.append(mk_pv)
                            step += 1
                            pump()
                    for fn in pend:
                        fn()
                    if hf == 1:
                        # phase-2 units must be EMITTED before normalize:
                        # engine deps are monotonic completion counters, so
                        # later emission inherits the whole normalize chain
                        phase = 2
                        pump()
                    normalize(n, hf, pys)
                    if hf == 0:
                        phase = 1
                    step += 1
                    pump()
            while done < len(inject):
                inject[done][1]()
                done += 1

        # ---- emission: k-major QKV(0) upfront (consumes per-k DMA pairs as
        # they land), then attn(n) with injected QKV(n+1) + proj(n-1) ----
        with nc.named_scope("qkv0"):
            acc_m = [ps.tile([128, CHW], f32, name=f"am{m}", tag=tg, bufs=2)
                     for m, tg in enumerate(("acc", "acc", "py", "py"))]
            accv = ps.tile([128, 2, CHW], f32, name="accv", tag="ss", bufs=2)
            for k in range(8):
                for m in range(4):
                    nc.tensor.matmul(
                        acc_m[m], wk[:, k, m * 128:(m + 1) * 128],
                        xts0[:, k, :], start=(k == 0), stop=(k == 7))
                for tt in range(4):
                    c0 = (tt % 2) * 256
                    # start=True clears has_written for the WHOLE bank, so
                    # only the first group per bank may start; the second
                    # group's k==0 write relies on that bank-wide clear
                    # (bit unset -> overwrite) and must never re-start.
                    nc.tensor.matmul(
                        accv[:, tt // 2, c0:c0 + 256],
                        xts0[:, k, tt * 128:(tt + 1) * 128],
                        wk[:, k, 512:768],
                        start=(k == 0 and tt % 2 == 0), stop=(k == 7),
                        skip_group_check=True)
            cols = slice(0, CHW)

            def evac_m(m):
                if m < 2:
                    nc.vector.tensor_scalar_add(
                        out=ktp_e[0:64, m, cols], in0=acc_m[m][0:64, :],
                        scalar1=bkq_s[0:64, m:m + 1])
                    nc.vector.tensor_scalar_add(
                        out=ktp_o[64:128, m, cols], in0=acc_m[m][64:128, :],
                        scalar1=bkq_s[64:128, m:m + 1])
                else:
                    nc.vector.tensor_scalar_add(
                        out=qsb[:, m - 2, cols], in0=acc_m[m],
                        scalar1=bkq_s[:, m:m + 1])

            def evac_v(tt):
                anch = vsb[:, (2 * tt) * 256:(2 * tt) * 256 + 1]
                dst = bass.AP(tensor=anch.tensor, offset=anch.offset,
                              ap=[anch.ap[0], [256, 2], [192, 2], [1, 64]])
                c0 = (tt % 2) * 256
                nc.vector.tensor_copy(
                    dst, accv[:, tt // 2, c0:c0 + 256].rearrange(
                        "p (hf sl d) -> p hf sl d", hf=2, sl=2))

            # DVE order matched to attn0's consumption: first QK needs m0
            # (kT) + m2 (q), first PV needs vT tiles 0/1
            evac_m(0)
            evac_m(2)
            evac_v(0)
            evac_v(1)
            evac_m(1)
            evac_m(3)
            evac_v(2)
            evac_v(3)

        def proj3_half(kk, os_):
            """Last chunk's proj split by head-pair: kk=0 (with bias) into
            outp as soon as hf=0 is normalized, kk=1 partial into outp2."""
            def emit():
                n = NCH - 1
                with nc.named_scope(f"proj{n}"):
                    for i, o in enumerate(os_):
                        tg = ("acc", "py")[i % 2] if kk == 1 else "acc"
                        acc = ps.tile([128, CHW], f32, name="acc", tag=tg,
                                      bufs=2)
                        nc.tensor.matmul(
                            acc, wp[:, kk, o * 128:(o + 1) * 128],
                            ysb[:, kk, n * CHW:(n + 1) * CHW],
                            start=True, stop=True)
                        ot = osp.tile([128, CHW], bf16, name="ot")
                        if kk == 0:
                            nc.vector.tensor_scalar_add(
                                out=ot, in0=acc, scalar1=bp_s[:, o:o + 1])
                            nc.sync.dma_start(
                                outp[o * 128:(o + 1) * 128,
                                     n * CHW:(n + 1) * CHW], ot)
                        else:
                            # alternate evac engines so the tail drains in
                            # parallel instead of serializing on ACT
                            if o % 2:
                                nc.scalar.copy(out=ot, in_=acc)
                            else:
                                nc.vector.tensor_copy(ot, acc)
                            nc.sync.dma_start(
                                outp2[o * 128:(o + 1) * 128, :], ot)
            return emit

        for n in range(NCH):
            inject = []
            if n >= 1:
                inject.append((0, proj_unit(n - 1, range(0, 4))))
                inject.append((0, proj_unit(n - 1, range(4, 8))))
            if n + 1 < NCH:
                inject.extend((0, qkv_m_unit(n + 1, m)) for m in range(2))
                if n + 2 < NCH:
                    inject.append((0, prefetch_x(n + 2)))
                inject.extend((0, qkv_m_unit(n + 1, m)) for m in range(2, 4))
                inject.extend((0, qkv_v_unit(n + 1, p)) for p in range(2))
            if n == NCH - 1:
                # last proj(2) unit reserved for the post-normalize(3,hf1)
                # window: it fills the PE gap while proj3b waits on ysb and
                # keeps the HAM clock from throttling through the tail
                inject = [inject[0], (1, proj3_half(0, range(0, 4))),
                          (1, proj3_half(0, range(4, 8))),
                          (2, inject[1][1])]
            attn_chunk(n, inject)

        proj3_half(1, range(8))()

    nc.compile()
    return nc


def _host_inputs(x, W_kqv, b_kqv, W_proj, b_proj):
    import ml_dtypes

    bf16 = ml_dtypes.bfloat16
    x = np.asarray(x, dtype=np.float32)
    W_kqv = np.asarray(W_kqv, dtype=np.float32)
    b_kqv = np.asarray(b_kqv, dtype=np.float32)
    W_proj = np.asarray(W_proj, dtype=np.float32)
    b_proj = np.asarray(b_proj, dtype=np.float32)

    ss, tt = np.meshgrid(np.arange(128), np.arange(128), indexing="ij")
    amask = (ss <= tt).astype(bf16)  # 0/1 multiplicative mask
    zpad = np.zeros((64, 2 * T), dtype=bf16)
    ones = np.ones((128, 4096), dtype=bf16)

    xts = [np.ascontiguousarray(x[b].T).astype(bf16) for b in range(B)]

    in_maps = []
    for c in range(NCORES):
        b, g = c // 4, c % 4
        heads = [4 * g + i for i in range(HPC)]
        wl = np.concatenate(
            [W_kqv[h * 192:h * 192 + 64] for h in heads]
            + [W_kqv[h * 192 + 64:h * 192 + 128] * 0.125 for h in heads]
            + [W_kqv[h * 192 + 128:h * 192 + 192] for h in heads], axis=0)
        bkql = np.concatenate(
            [b_kqv[h * 192:h * 192 + 64] for h in heads]
            + [b_kqv[h * 192 + 64:h * 192 + 128] * 0.125 for h in heads])
        wp_g = W_proj[:, 256 * g:256 * (g + 1)]
        bv_g = np.concatenate(
            [b_kqv[h * 192 + 128:h * 192 + 192] for h in heads])
        bp_eff = (b_proj if g == 0 else np.zeros_like(b_proj)) + wp_g @ bv_g
        in_maps.append({
            "xt": xts[b],
            "wkqv": np.ascontiguousarray(wl.T).astype(bf16),
            "bkq": np.ascontiguousarray(bkql.reshape(4, 128).T,
                                        dtype=np.float32),
            "wproj": np.ascontiguousarray(wp_g.T).astype(bf16),
            "bp": np.ascontiguousarray(bp_eff.reshape(8, 128).T,
                                       dtype=np.float32),
            "amask": amask,
            "zpad": zpad,
            "ones": ones,
        })
    return in_maps


def kernel(x, W_kqv, b_kqv, W_proj, b_proj):
    from concourse.bass_utils import run_bass_kernel_spmd

    if "nc" not in _cache:
        _cache["nc"] = _build_nc()
    nc = _cache["nc"]

    in_maps = _host_inputs(x, W_kqv, b_kqv, W_proj, b_proj)
    trace = bool(int(os.environ.get("KERNEL_TRACE", "0")))
    r = run_bass_kernel_spmd(nc, in_maps, core_ids=list(range(NCORES)),
                             trace=trace)
    if trace:
        _cache["last_results"] = r
        print(f"HW exec time: {r.exec_time_ns} ns")

    out = np.empty((B, T, C), dtype=np.float32)
    for b in range(B):
        acc = np.zeros((C, T), dtype=np.float32)
        for g in range(4):
            acc += r.results[4 * b + g]["outp"].astype(np.float32)
            acc[:, T - CHW:] += r.results[4 * b + g]["outp2"].astype(
                np.float32)
        out[b] = acc.T
    return out
